# revision 30
# baseline (speedup 1.0000x reference)
"""Multi-head attention on 8 Trainium2 NeuronCores — JIT-woven pipeline v2.

Problem: x[4, 2048, 1024], 16 heads x 64 dim.
  qkv = x @ w_qkv; attn = softmax(q k^T / 8); out = (attn v) @ w_out + b_out

Sharding: 8 cores = 4 batches x 2 head-groups (8 heads each); host sums
the two partial out-projections per batch and adds the bias.

v2 vs v1: the v1 kernel ran phase 1 (qkv projections) as a solo boot
block before the cell stream, so a single execution (what the harness
times) serialized ~120 us of TensorE work against an idle ScalarE, then
ran the cell stream ScalarE-bound with an idle TensorE surplus. v2
emits phase-1 units just-in-time inside the cell stream:

  - cell streams are p-major (pair outer, i-chunk inner) so each pair's
    k/v projections are first needed at its own block, spreading the
    phase-1 load across the whole rep;
  - a dependency tracker force-emits a cell's phase-1 units right
    before the cell, and a slack-credit weaver prefetches future units
    into per-cell TensorE slack (ScalarE exp is the cell-stream pacer);
  - the fp8 DoubleRow repack of q/k is done by direct Pool/DVE engine
    copies from the projection's PSUM tile (4 partition-shifted copies)
    instead of v1's SBUF stage + SP-HWDGE DMA round trip, making
    repacked slices available ~1 us after the projection instead of ~8;
  - x chunks stay resident all rep (8 distinct tiles) since p-major
    ordering touches every chunk in every block; qk8/v/aoT tiles are
    keyed per (pair/j-tile/i-chunk) so bufs=1 still gives cross-rep
    overlap via region-exact WAR deps;
  - scores run as fp8e4m3 DoubleRow matmuls; softmax/AV stay bf16 with
    fp32 PSUM; the AV matmul fuses the row-sum via a ones block.
"""

from collections import deque

import numpy as np

import concourse.bacc as bacc
import concourse.mybir as mybir
import concourse.tile as tile
from concourse.bass_utils import run_bass_kernel_spmd

F32 = mybir.dt.float32
BF16 = mybir.dt.bfloat16
F8 = mybir.dt.float8e4
PM = mybir.MatmulPerfMode.DoubleRow
AF = mybir.ActivationFunctionType

B = 4          # batch
N = 2048       # sequence
DM = 1024      # model dim
NH = 16        # heads
DH = 64        # head dim
G = 2          # head groups (cores per batch)
HPC = NH // G  # heads per core = 8
CW = DH * HPC  # per-core qkv column width = 512
NP = HPC // 2  # head pairs per core = 4

NCH = 256      # x column chunk (sequence positions per chunk)
ICH = 512      # i (query) chunk (per head; a pair shares [128, 2*ICH])

KT = DM // 128      # 8 contraction tiles over d
NXC = N // NCH      # 8 x chunks
NJT = N // 128      # 16 j tiles
NIC = N // ICH      # 4 i chunks
NTPI = ICH // 128   # 4 n-tiles per i chunk

SLACK = 250.0       # ns of TensorE slack banked per cell for filler work
CREDIT_CAP = 1500.0  # caps filler bursts that would starve the exp chain

# repack mode per qk8 m-tile: "pool"/"dve" put all 4 quadrant copies on one
# engine; "split" puts partition-half hh=0 on Pool and hh=1 on DVE (disjoint
# partition ranges, so no cross-engine write-ordering serialization)
REPACK_ENGINES = ("pool", "pool", "pool", "pool",
                  "split", "split", "split", "split")

# "" = all projections bf16; "k" = key projections via fp8e4m3 DoubleRow
# matmuls (host-packed fp8 x and w_k) — k gets re-quantized to fp8 after
# the projection anyway, so the extra input-quantization noise is modest
FP8_PROJ = "k"


def build_nc(reps=1, slack=SLACK, credit_cap=CREDIT_CAP,
             repack_engines=REPACK_ENGINES, pv_engine="gpsimd", spacer=1,
             fp8_proj=FP8_PROJ):
    nc = bacc.Bacc(None, target_bir_lowering=False, debug=False)

    # weights are host-packed tile-major ([partition, k-tile, col]) so each
    # weight tensor loads in 1-2 large DMAs instead of 8+ small ones — the
    # cold start is bounded by per-DMA pipeline latency, not bytes
    WQK_W = CW if fp8_proj == "k" else 2 * CW
    xT = nc.declare_dram_parameter("xT", [NXC, 128, KT * NCH], BF16,
                                   isOutput=False)
    wqk = nc.declare_dram_parameter("wqk", [128, KT * WQK_W], BF16,
                                    isOutput=False)
    if fp8_proj == "k":
        # q columns only in bf16; k columns DoubleRow-packed fp8
        wqk8 = nc.declare_dram_parameter("wqk8", [64, KT * 2 * CW], F8,
                                         isOutput=False)
        xT8 = nc.declare_dram_parameter("xT8", [NXC, 64, KT * 2 * NCH], F8,
                                        isOutput=False)
    wv = nc.declare_dram_parameter("wv", [128, KT * CW], BF16, isOutput=False)
    wo = nc.declare_dram_parameter("wo", [128, (CW // 128) * DM], BF16,
                                   isOutput=False)
    out = nc.declare_dram_parameter("out", [N, DM], F32, isOutput=True)

    with tile.TileContext(nc) as tc:
        with (
            tc.tile_pool(name="cpool", bufs=1) as cpool,
            # 8 PSUM banks: "s" 2x[128,1024] scores, "av" 1x[128,1024]
            # accumulator, "p1" 2x[128,512] projections
            tc.tile_pool(name="psA", bufs=2, space="PSUM") as psA,
            tc.tile_pool(name="psB", bufs=1, space="PSUM") as psB,
            tc.tile_pool(name="psC", bufs=2, space="PSUM") as psC,
            tc.tile_pool(name="epool", bufs=5) as epool,
            tc.tile_pool(name="wpool", bufs=1) as wpool,
            tc.tile_pool(name="lpool", bufs=2) as lpool,
            tc.tile_pool(name="xpool", bufs=1) as xpool,
            tc.tile_pool(name="aopool", bufs=1) as aopool,
        ):
            st = {}  # per-rep tile sets

            def emit_alloc(r):
                st[r] = {
                    # DoubleRow-packed q (m 0-3) / k (m 4-7): [64, 2, N] fp8,
                    # pair's heads at partition offsets 0/32, d = p + 32*plane
                    "qk8": [cpool.tile([64, 2, N], F8, name=f"qk8_{m}")
                            for m in range(2 * NP)],
                    # per pair: 16 j-tile blocks of [v_even | ones | v_odd];
                    # one strided memset per pair writes all 16 ones blocks
                    # (64 tiny memsets would flood the Pool queue)
                    "v": [cpool.tile([128, NJT * 3 * DH], BF16,
                                     name=f"v_{c}") for c in range(NP)],
                    "x": {},
                    "aoT": {},
                }
                for c in range(NP):
                    v3 = st[r]["v"][c].rearrange("p (j x) -> p j x",
                                                 x=3 * DH)
                    nc.gpsimd.memset(v3[:, :, DH:2 * DH], 1.0)

            def emit_dma_wqk(r):
                # two half DMAs across both queues at cold start; one sem
                # per half so 4 k-tiles arrive at once
                s = st[r]
                s["wqkT"] = wpool.tile([128, KT * WQK_W], BF16, name="wqkT")
                halfw = KT * WQK_W // 2
                nc.sync.dma_start(s["wqkT"][:, 0:halfw], wqk[:, 0:halfw])
                eng = nc.scalar if r == 0 else nc.sync
                eng.dma_start(s["wqkT"][:, halfw:], wqk[:, halfw:])

            def emit_dma_wqk8(r):
                s = st[r]
                s["wqk8T"] = wpool.tile([64, KT * 2 * CW], F8, name="wqk8T")
                eng = nc.scalar if r == 0 else nc.sync
                eng.dma_start(s["wqk8T"][:], wqk8[:])

            KARR = tuple(range(KT))

            def emit_dma_wv(r):
                s = st[r]
                s["wvT"] = wpool.tile([128, KT * CW], BF16, name="wvT")
                eng = nc.scalar if r == 0 else nc.sync
                eng.dma_start(s["wvT"][:], wv[:])

            def emit_dma_wo(r):
                # deferred past the cold-critical tensors: wo is first read
                # ~60+ us in, and the DMA engines serialize all transfers
                s = st[r]
                s["woT"] = wpool.tile([128, (CW // 128) * DM], BF16,
                                      name="woT")
                nc.sync.dma_start(s["woT"][:], wo[:])

            def emit_dma_x(r, ch):
                x_t = xpool.tile([128, KT * NCH], BF16, name=f"x{ch}")
                if ch == 0:
                    # split the first chunk across both HWDGE queues so the
                    # first projection's k-tiles land ~2x sooner
                    half = KT * NCH // 2
                    nc.sync.dma_start(x_t[:, 0:half], xT[ch][:, 0:half])
                    nc.scalar.dma_start(x_t[:, half:], xT[ch][:, half:])
                else:
                    nc.sync.dma_start(x_t[:], xT[ch])
                st[r]["x"][ch] = x_t

            def emit_dma_x8(r, ch):
                x8_t = xpool.tile([64, KT * 2 * NCH], F8, name=f"x8{ch}")
                nc.sync.dma_start(x8_t[:], xT8[ch])
                st[r]["x8"] = st[r].get("x8", {})
                st[r]["x8"][ch] = x8_t

            def emit_pq(r, m, ch):
                """q|k projection for m-tile m, x chunk ch: [128, 256] PSUM,
                then 4 partition-shifted copies into DoubleRow layout."""
                s = st[r]
                pq = psC.tile([128, NCH], F32, name="pq", tag="p1")
                if fp8_proj == "k" and m >= NP:
                    # fp8 DoubleRow: contraction 128 = 64 partitions x 2
                    # planes (d = p + 64t per k-tile), half the PE cycles
                    x8v = s["x8"][ch].rearrange("p (k t n) -> p k t n",
                                                k=KT, t=2)
                    w8v = s["wqk8T"].rearrange("p (k t c) -> p k t c",
                                               k=KT, t=2)
                    msl = slice((m - NP) * 128, (m - NP + 1) * 128)
                    for i, k in enumerate(KARR):
                        nc.tensor.matmul(
                            pq[:],
                            w8v[:, k, :, msl],
                            x8v[:, k, :, :],
                            start=(i == 0), stop=(i == KT - 1),
                            perf_mode=PM,
                        )
                else:
                    x_t = s["x"][ch]
                    for i, k in enumerate(KARR):
                        nc.tensor.matmul(
                            pq[:],
                            s["wqkT"][:, k * WQK_W + m * 128:
                                      k * WQK_W + (m + 1) * 128],
                            x_t[:, k * NCH:(k + 1) * NCH],
                            start=(i == 0), stop=(i == KT - 1),
                        )
                # GPSIMD cannot touch PSUM, so: one DVE cast to an fp8 SBUF
                # stage, then 4 partition-shifted SBUF->SBUF copies scatter
                # it into the DoubleRow layout, split across Pool and DVE
                # per the repack_engines mode for this m-tile
                stage = lpool.tile([128, NCH], F8, name="qs", tag="qs",
                                   bufs=4)
                nc.vector.tensor_copy(stage[:], pq[:])
                qk = s["qk8"][m]
                csl = slice(ch * NCH, (ch + 1) * NCH)
                mode = repack_engines[m]
                for hh in range(2):
                    if mode == "split":
                        eng = nc.gpsimd if hh == 0 else nc.vector
                    else:
                        eng = nc.gpsimd if mode == "pool" else nc.vector
                    for t in range(2):
                        eng.tensor_copy(
                            qk[32 * hh:32 * hh + 32, t, csl],
                            stage[hh * DH + 32 * t:hh * DH + 32 * (t + 1), :],
                        )

            def emit_pv(r, ch, mt, c):
                """v projection for j-tile 2ch+mt, pair c: [128, 128] PSUM,
                copied around the ones block of the pair's v tile."""
                s = st[r]
                x_t = s["x"][ch]
                j = ch * (NCH // 128) + mt
                pv = psC.tile([128, 128], F32, name="pv", tag="p1")
                for k in range(KT):
                    nc.tensor.matmul(
                        pv[:],
                        x_t[:, k * NCH + mt * 128:k * NCH + (mt + 1) * 128],
                        s["wvT"][:, k * CW + c * 128:k * CW + (c + 1) * 128],
                        start=(k == 0), stop=(k == KT - 1),
                    )
                base = j * 3 * DH
                vt = s["v"][c]
                nc.vector.tensor_copy(vt[:, base:base + DH], pv[:, 0:DH])
                nc.vector.tensor_copy(vt[:, base + 2 * DH:base + 3 * DH],
                                      pv[:, DH:2 * DH])

            # ---------------- cell-stream emission ----------------
            def emit_scores(r, p, ic, jt):
                s = st[r]
                isl = slice(ic * ICH, (ic + 1) * ICH)
                s_ps = psA.tile([128, 2 * ICH], F32, name="s_ps", tag="s")
                for half in range(2):
                    po = 32 * half
                    q8 = s["qk8"][p]
                    k8 = s["qk8"][NP + p]
                    nc.tensor.matmul(
                        s_ps[:, half * ICH:(half + 1) * ICH],
                        k8[po:po + 32, :, jt * 128:(jt + 1) * 128],
                        q8[po:po + 32, :, isl],
                        start=True, stop=True,
                        perf_mode=PM,
                    )
                ex = epool.tile([128, 2 * ICH], BF16, name="ex", tag="ex")
                nc.scalar.activation(ex[:], s_ps[:], AF.Exp, scale=0.125)
                return ex

            def emit_av(r, p, jt, ex, av2):
                s = st[r]
                for half in range(2):
                    base = jt * 3 * DH + half * DH
                    vl = s["v"][p][:, base:base + 2 * DH]
                    nc.tensor.matmul(
                        av2[:, half * ICH:(half + 1) * ICH],
                        vl,
                        ex[:, half * ICH:(half + 1) * ICH],
                        start=(jt == 0), stop=(jt == NJT - 1),
                    )

            def emit_normalize(r, p, ic, av2, last=False):
                s = st[r]
                # one fast copy releases the PSUM accumulator for the next
                # stream; the reciprocal/mul chain runs off-critical-path.
                # For the rep's last stream no one needs the bank: skip the
                # copy and normalize straight from PSUM.
                if last:
                    avs = av2
                else:
                    avs = lpool.tile([128, 2 * ICH], F32, name="avs",
                                     tag="avs", bufs=2)
                    nc.vector.tensor_copy(avs[:], av2[:])
                for half in range(2):
                    l = 2 * p + half
                    ct, coff = l // 2, (l % 2) * DH
                    ao = s["aoT"].get((ct, ic))
                    if ao is None:
                        ao = aopool.tile([128, ICH], BF16,
                                         name=f"aoT{ct}_{ic}")
                        s["aoT"][(ct, ic)] = ao
                    # even head: rows [out | sums]; odd head: [sums | out]
                    o0, s0 = (0, DH) if half == 0 else (DH, 0)
                    hsl = slice(half * ICH, (half + 1) * ICH)
                    # rc rows sit at the out-rows' base partition: the DVE
                    # mul requires equal base partitions for two SBUF inputs.
                    # The recip/mul chain runs on GPSIMD (all-SBUF, legal
                    # there) to unload DVE — except last=True, where avs is
                    # still in PSUM, which GPSIMD cannot read.
                    eng = nc.vector if last else nc.gpsimd
                    rc = lpool.tile([128, ICH], F32, name="rc", tag="rc",
                                    bufs=2)
                    nc.vector.reciprocal(rc[o0:o0 + DH, :],
                                         avs[s0:s0 + DH, hsl])
                    eng.tensor_mul(
                        ao[coff:coff + DH, :],
                        avs[o0:o0 + DH, hsl],
                        rc[o0:o0 + DH, :],
                    )

            def emit_ph3(r, ic, lnt, h, drain=False):
                s = st[r]
                nt = ic * NTPI + lnt
                po = psC.tile([128, 512], F32, name="po", tag="p1")
                for c in range(CW // 128):
                    nc.tensor.matmul(
                        po[:],
                        s["aoT"][(c, ic)][:, lnt * 128:(lnt + 1) * 128],
                        s["woT"][:, c * DM + h * 512:c * DM + (h + 1) * 512],
                        start=(c == 0), stop=(c == CW // 128 - 1),
                    )
                os_ = lpool.tile([128, 512], F32, name="os", tag="os",
                                 bufs=3)
                if drain:
                    # tail units: the exp stream is done, so the idle ACT
                    # engine does the PSUM->SBUF copy (Copy shares the exp
                    # act table — no reload)
                    nc.scalar.activation(os_[:], po[:], AF.Copy)
                else:
                    nc.vector.tensor_copy(os_[:], po[:])
                nc.sync.dma_start(
                    out[nt * 128:(nt + 1) * 128, h * 512:(h + 1) * 512],
                    os_[:],
                )

            # ---------------- background unit queue ----------------
            def cell_order():
                # p-major with per-block i-chunk rotation: ph3(ic) needs all
                # pairs' norms at ic; rotation staggers each ic's completion
                # across block boundaries instead of piling all out-projection
                # work after the last block
                return [(p, (ic + p) % NIC, jt) for p in range(NP)
                        for ic in range(NIC) for jt in range(NJT)]

            def cell_deps(r, p, ic, jt):
                # pv last: its matmuls wait on wvT, which arrives after the
                # score-critical tensors — don't head-of-line block the q/k
                # units behind it in the PE queue
                return [
                    (r, "pq", p, 2 * ic),
                    (r, "pq", NP + p, jt // 2),
                    (r, "pq", p, 2 * ic + 1),
                    (r, "pv", jt // 2, jt % 2, p),
                ]

            def build_queue(r):
                """Background units for rep r in first-need order.
                Entry: (cost_ns, fn, key)."""
                q = deque()
                queued = set()

                def add(cost, fn, key):
                    if key in queued:
                        return
                    queued.add(key)
                    q.append((cost, fn, key))

                def need_x(ch):
                    for c2 in range(min(ch + 2, NXC - 1) + 1):
                        add(0.0, (lambda c3=c2: emit_dma_x(r, c3)),
                            (r, "x", c2))
                        if fp8_proj == "k":
                            add(0.0, (lambda c3=c2: emit_dma_x8(r, c3)),
                                (r, "x8", c2))

                add(0.0, (lambda: emit_alloc(r)), (r, "alloc"))
                add(0.0, (lambda: emit_dma_x(r, 0)), (r, "x", 0))
                add(0.0, (lambda: emit_dma_wqk(r)), (r, "wqk"))
                if fp8_proj == "k":
                    add(0.0, (lambda: emit_dma_x8(r, 0)), (r, "x8", 0))
                    add(0.0, (lambda: emit_dma_wqk8(r)), (r, "wqk8"))
                add(0.0, (lambda: emit_dma_x(r, 1)), (r, "x", 1))
                add(0.0, (lambda: emit_dma_wv(r)), (r, "wv"))
                for (p, ic, jt) in cell_order():
                    if True:
                        if (p, jt) == (0, 8):
                            add(0.0, (lambda: emit_dma_wo(r)), (r, "wo"))
                        if True:
                            for key in cell_deps(r, p, ic, jt):
                                if key in queued:
                                    continue
                                kind = key[1]
                                if kind == "pq":
                                    _, _, m, ch = key
                                    need_x(ch)
                                    cost = (427.0 if fp8_proj == "k"
                                            and m >= NP else 853.0)
                                    add(cost,
                                        (lambda m=m, ch=ch:
                                         emit_pq(r, m, ch)), key)
                                else:
                                    _, _, ch, mt, c = key
                                    need_x(ch)
                                    add(427.0,
                                        (lambda ch=ch, mt=mt, c=c:
                                         emit_pv(r, ch, mt, c)), key)
                return q

            emitted = set()

            def emit_cells(r, fillers, ph3q, is_last_rep=False):
                """Cell stream for rep r. fillers: background queue
                (leftovers from rep r-1 then rep r+1's units). ph3q:
                out-projection units, popped with priority."""
                credit = 0.0

                def pop_filler(force=False):
                    nonlocal credit
                    if force:
                        q = fillers if fillers else ph3q
                    else:
                        q = ph3q if ph3q else fillers
                    if not q:
                        return False
                    cost = q[0][0]
                    if not force and credit < cost:
                        return False
                    ent = q.popleft()
                    ent[1]()
                    if len(ent) > 2:
                        emitted.add(ent[2])
                    if not force:
                        credit -= cost
                    return True

                def ensure(key):
                    while key not in emitted:
                        assert fillers, f"dep {key} not in queue"
                        ent = fillers.popleft()
                        ent[1]()
                        if len(ent) > 2:
                            emitted.add(ent[2])

                def pop_free():
                    # zero-cost entries (DMAs/allocs) at the front ride free
                    while fillers and fillers[0][0] == 0.0:
                        ent = fillers.popleft()
                        ent[1]()
                        if len(ent) > 2:
                            emitted.add(ent[2])

                cells = cell_order()
                av_cur = None
                pend = None  # (p, ic, jt, ex, av2)
                norm_done = {}

                def finish(cell):
                    nonlocal credit
                    p, ic, jt, ex, av2 = cell
                    emit_av(r, p, jt, ex, av2)
                    credit = min(credit + slack, credit_cap)
                    if jt == NJT - 1:
                        emit_normalize(r, p, ic, av2,
                                       last=(is_last_rep
                                             and (p, ic) == cells[-1][:2]))
                        norm_done[ic] = norm_done.get(ic, 0) + 1
                        if norm_done[ic] == NP:
                            drain = (is_last_rep
                                     and (p, ic) == cells[-1][:2])
                            for lnt in range(NTPI):
                                for h in range(DM // 512):
                                    ph3q.append(
                                        (853.0,
                                         lambda ic=ic, lnt=lnt, h=h,
                                         drain=drain:
                                         emit_ph3(r, ic, lnt, h, drain)))
                        # PE spacer while DVE copies the accumulator out
                        for _ in range(spacer):
                            pop_filler(force=True)
                    else:
                        pop_free()
                        while pop_filler():
                            pass

                for i, (p, ic, jt) in enumerate(cells):
                    if jt == 0:
                        av_cur = psB.tile([128, 2 * ICH], F32, name="av",
                                          tag="av")
                    for key in cell_deps(r, p, ic, jt):
                        ensure(key)
                    ex = emit_scores(r, p, ic, jt)
                    if pend is not None:
                        finish(pend)
                    pend = (p, ic, jt, ex, av_cur)
                    if jt == 10 and i + 6 < len(cells):
                        # force-prefetch the next stream's deps so its first
                        # scores don't stall on just-in-time projections and
                        # their repack copies at the stream boundary
                        for key in cell_deps(r, *cells[i + 6]):
                            ensure(key)
                finish(pend)
                return fillers, ph3q

            # ---------------- weave reps ----------------
            carry = deque()
            ph3q = deque()
            for r in range(reps):
                fillers = deque(carry)
                fillers.extend(build_queue(r))
                carry, ph3q = emit_cells(r, fillers, ph3q,
                                         is_last_rep=(r == reps - 1))
            while ph3q or carry:
                q = ph3q if ph3q else carry
                q.popleft()[1]()

    nc.finalize()
    return nc


def make_in_maps(inputs_np, fp8_proj=FP8_PROJ):
    bf16 = mybir.dt.np(BF16)
    f8 = mybir.dt.np(F8)
    x = np.ascontiguousarray(inputs_np["x"], dtype=np.float32)
    w_qkv = np.asarray(inputs_np["w_qkv"], dtype=np.float32)
    w_out = np.asarray(inputs_np["w_out"], dtype=np.float32)
    in_maps = []
    xp_cache = {}
    x8_cache = {}
    for core in range(8):
        b, g = divmod(core, 2)
        if b not in xp_cache:
            # pack x[b]^T as [chunk, partition, k, n] so device chunk loads
            # are single linear DMAs
            xp_cache[b] = np.ascontiguousarray(
                x[b].T.reshape(KT, 128, NXC, NCH).transpose(2, 1, 0, 3)
            ).reshape(NXC, 128, KT * NCH).astype(bf16)
            if fp8_proj == "k":
                # DoubleRow pack: partition p, plane t hold d = p + 64t per
                # k-tile -> [ch, 64, (k, t, n)]
                x8_cache[b] = np.ascontiguousarray(
                    x[b].T.reshape(KT, 2, 64, NXC, NCH)
                    .transpose(3, 2, 0, 1, 4)
                ).reshape(NXC, 64, KT * 2 * NCH).astype(f8)
        xTb = xp_cache[b]
        wq = w_qkv[:, g * CW:(g + 1) * CW]
        wk = w_qkv[:, DM + g * CW:DM + (g + 1) * CW]
        wv_ = w_qkv[:, 2 * DM + g * CW:2 * DM + (g + 1) * CW]

        def tilemajor(w):  # [DM, C] -> [128, KT*C], cols (k, c)
            c = w.shape[1]
            return np.ascontiguousarray(
                w.reshape(KT, 128, c).transpose(1, 0, 2)
            ).reshape(128, KT * c)

        wo_ = w_out[g * CW:(g + 1) * CW, :]
        im = {
            "xT": xTb,
            "wv": tilemajor(wv_).astype(bf16),
            "wo": np.ascontiguousarray(
                wo_.reshape(CW // 128, 128, DM).transpose(1, 0, 2)
            ).reshape(128, (CW // 128) * DM).astype(bf16),
        }
        if fp8_proj == "k":
            im["wqk"] = tilemajor(wq).astype(bf16)
            # DoubleRow pack of w_k: [64, (k, t, c)], d = k*128 + 64t + p
            im["wqk8"] = np.ascontiguousarray(
                wk.reshape(KT, 2, 64, CW).transpose(2, 0, 1, 3)
            ).reshape(64, KT * 2 * CW).astype(f8)
            im["xT8"] = x8_cache[b]
        else:
            im["wqk"] = tilemajor(
                np.concatenate([wq, wk], axis=1)).astype(bf16)
        in_maps.append(im)
    return in_maps


_NC_CACHE = {}


def _get_nc():
    if "nc" not in _NC_CACHE:
        _NC_CACHE["nc"] = build_nc()
    return _NC_CACHE["nc"]


def kernel(x, w_qkv, w_out, b_out):
    b_out = np.asarray(b_out, dtype=np.float32)
    nc = _get_nc()
    in_maps = make_in_maps({"x": x, "w_qkv": w_qkv, "w_out": w_out})
    res = run_bass_kernel_spmd(nc, in_maps, core_ids=list(range(8)))
    _NC_CACHE["last_result"] = res
    out = np.empty((B, N, DM), np.float32)
    for b in range(B):
        out[b] = res.results[2 * b]["out"] + res.results[2 * b + 1]["out"] + b_out
    return out


# revision 43
# speedup vs baseline: 1.1770x; 1.1770x over previous
"""Multi-head attention on 8 Trainium2 NeuronCores — JIT-woven pipeline v2.

Problem: x[4, 2048, 1024], 16 heads x 64 dim.
  qkv = x @ w_qkv; attn = softmax(q k^T / 8); out = (attn v) @ w_out + b_out

Sharding: 8 cores = 4 batches x 2 head-groups (8 heads each); host sums
the two partial out-projections per batch and adds the bias.

v2 vs v1: the v1 kernel ran phase 1 (qkv projections) as a solo boot
block before the cell stream, so a single execution (what the harness
times) serialized ~120 us of TensorE work against an idle ScalarE, then
ran the cell stream ScalarE-bound with an idle TensorE surplus. v2
emits phase-1 units just-in-time inside the cell stream:

  - cell streams are p-major (pair outer, i-chunk inner) so each pair's
    k/v projections are first needed at its own block, spreading the
    phase-1 load across the whole rep;
  - a dependency tracker force-emits a cell's phase-1 units right
    before the cell, and a slack-credit weaver prefetches future units
    into per-cell TensorE slack (ScalarE exp is the cell-stream pacer);
  - the fp8 DoubleRow repack of q/k is done by direct Pool/DVE engine
    copies from the projection's PSUM tile (4 partition-shifted copies)
    instead of v1's SBUF stage + SP-HWDGE DMA round trip, making
    repacked slices available ~1 us after the projection instead of ~8;
  - x chunks stay resident all rep (8 distinct tiles) since p-major
    ordering touches every chunk in every block; qk8/v/aoT tiles are
    keyed per (pair/j-tile/i-chunk) so bufs=1 still gives cross-rep
    overlap via region-exact WAR deps;
  - scores run as fp8e4m3 DoubleRow matmuls; softmax/AV stay bf16 with
    fp32 PSUM; the AV matmul fuses the row-sum via a ones block.
"""

from collections import deque

import numpy as np

import concourse.bacc as bacc
import concourse.mybir as mybir
import concourse.tile as tile
from concourse.bass_utils import run_bass_kernel_spmd

F32 = mybir.dt.float32
BF16 = mybir.dt.bfloat16
F8 = mybir.dt.float8e4
PM = mybir.MatmulPerfMode.DoubleRow
AF = mybir.ActivationFunctionType

B = 4          # batch
N = 2048       # sequence
DM = 1024      # model dim
NH = 16        # heads
DH = 64        # head dim
G = 2          # head groups (cores per batch)
HPC = NH // G  # heads per core = 8
CW = DH * HPC  # per-core qkv column width = 512
NP = HPC // 2  # head pairs per core = 4

NCH = 256      # x column chunk (sequence positions per chunk)
ICH = 512      # i (query) chunk (per head; a pair shares [128, 2*ICH])

KT = DM // 128      # 8 contraction tiles over d
NXC = N // NCH      # 8 x chunks
NJT = N // 128      # 16 j tiles
NIC = N // ICH      # 4 i chunks
NTPI = ICH // 128   # 4 n-tiles per i chunk

SLACK = 250.0       # ns of TensorE slack banked per cell for filler work
CREDIT_CAP = 1500.0  # caps filler bursts that would starve the exp chain

# repack mode per qk8 m-tile: "pool"/"dve" put all 4 quadrant copies on one
# engine; "split" puts partition-half hh=0 on Pool and hh=1 on DVE (disjoint
# partition ranges, so no cross-engine write-ordering serialization)
REPACK_ENGINES = ("pool", "pool", "pool", "pool",
                  "split", "split", "split", "split")

# Note: an fp8e4m3-DoubleRow variant of the k projections was measured to
# save ~14 us of TensorE and ~6 us of steady-state span, but cost 2 MB of
# extra cold DMA traffic (slower single-shot) and raised rel err
# 1.25e-2 -> 1.73e-2 against the 2e-2 gate, so projections stay bf16.


def build_nc(reps=1, slack=SLACK, credit_cap=CREDIT_CAP,
             repack_engines=REPACK_ENGINES, pv_engine="gpsimd", spacer=1):
    nc = bacc.Bacc(None, target_bir_lowering=False, debug=False)

    # weights are host-packed tile-major and split per head-pair: block p
    # only reads m-tiles {p, NP+p}, so each block's weight slice is one
    # small just-in-time DMA and the cold start moves the minimum bytes
    # before the first exp (the DMA engines serialize all transfers)
    xT = nc.declare_dram_parameter("xT", [NXC, 128, KT * NCH], BF16,
                                   isOutput=False)
    wqk = nc.declare_dram_parameter("wqk", [NP, 128, KT * 256], BF16,
                                    isOutput=False)
    wv = nc.declare_dram_parameter("wv", [NP, 128, KT * 128], BF16,
                                   isOutput=False)
    wo = nc.declare_dram_parameter("wo", [128, (CW // 128) * DM], BF16,
                                   isOutput=False)
    out = nc.declare_dram_parameter("out", [N, DM], F32, isOutput=True)

    with tile.TileContext(nc) as tc:
        with (
            tc.tile_pool(name="cpool", bufs=1) as cpool,
            # 8 PSUM banks: "s" 2x[128,1024] scores, "av" 1x[128,1024]
            # accumulator, "p1" 2x[128,512] projections
            tc.tile_pool(name="psA", bufs=2, space="PSUM") as psA,
            tc.tile_pool(name="psB", bufs=1, space="PSUM") as psB,
            tc.tile_pool(name="psC", bufs=2, space="PSUM") as psC,
            tc.tile_pool(name="epool", bufs=5) as epool,
            tc.tile_pool(name="wpool", bufs=1) as wpool,
            tc.tile_pool(name="lpool", bufs=2) as lpool,
            tc.tile_pool(name="xpool", bufs=1) as xpool,
            tc.tile_pool(name="aopool", bufs=1) as aopool,
        ):
            st = {}  # per-rep tile sets

            def emit_warmup():
                # PE p-state ramps to full clock only after ~3 us of
                # continuous busy; burn the DMA-bound cold window on dummy
                # matmuls so the first real projections run at 2.4 GHz
                wt = lpool.tile([128, DH], BF16, name="warm", bufs=1)
                nc.gpsimd.memset(wt[:], 0.0)
                wps = psB.tile([128, 2 * ICH], F32, name="av", tag="av")
                for i in range(60):
                    nc.tensor.matmul(wps[0:DH, 0:DH], wt[:], wt[:],
                                     start=(i == 0), stop=(i == 59))

            def emit_alloc(r):
                st[r] = {
                    # DoubleRow-packed q (m 0-3) / k (m 4-7): [64, 2, N] fp8,
                    # pair's heads at partition offsets 0/32, d = p + 32*plane
                    "qk8": [cpool.tile([64, 2, N], F8, name=f"qk8_{m}")
                            for m in range(2 * NP)],
                    # per pair: 16 j-tile blocks of [v_even | ones | v_odd];
                    # one strided memset per pair writes all 16 ones blocks
                    # (64 tiny memsets would flood the Pool queue)
                    "v": [cpool.tile([128, NJT * 3 * DH], BF16,
                                     name=f"v_{c}") for c in range(NP)],
                    "x": {},
                    "aoT": {},
                }
                for c in range(NP):
                    v3 = st[r]["v"][c].rearrange("p (j x) -> p j x",
                                                 x=3 * DH)
                    nc.gpsimd.memset(v3[:, :, DH:2 * DH], 1.0)

            KARR = tuple(range(KT))

            def emit_dma_wqk(r, p):
                # per-pair slice [k-tile, m_p cols | m_{NP+p} cols]
                s = st[r]
                t = wpool.tile([128, KT * 256], BF16, name=f"wqkP{p}")
                s.setdefault("wqkP", {})[p] = t
                nc.sync.dma_start(t[:], wqk[p])

            def emit_dma_wv(r, p):
                s = st[r]
                t = wpool.tile([128, KT * 128], BF16, name=f"wvP{p}")
                s.setdefault("wvP", {})[p] = t
                eng = nc.scalar if (r == 0 and p == 0) else nc.sync
                eng.dma_start(t[:], wv[p])

            def emit_dma_wo(r):
                # deferred past the cold-critical tensors: wo is first read
                # ~60+ us in, and the DMA engines serialize all transfers
                s = st[r]
                s["woT"] = wpool.tile([128, (CW // 128) * DM], BF16,
                                      name="woT")
                nc.sync.dma_start(s["woT"][:], wo[:])

            def emit_dma_x(r, ch):
                x_t = xpool.tile([128, KT * NCH], BF16, name=f"x{ch}")
                if r == 0 and ch == 0:
                    # split the first chunk across both HWDGE queues so the
                    # first projection's k-tiles land ~2x sooner
                    half = KT * NCH // 2
                    nc.sync.dma_start(x_t[:, 0:half], xT[ch][:, 0:half])
                    nc.scalar.dma_start(x_t[:, half:], xT[ch][:, half:])
                elif r == 0 and ch == 1:
                    # scalar queue: round-robins right after wqkP0 on sync,
                    # ahead of the x2+ prefetches
                    nc.scalar.dma_start(x_t[:], xT[ch])
                else:
                    nc.sync.dma_start(x_t[:], xT[ch])
                st[r]["x"][ch] = x_t

            def emit_pq(r, m, ch):
                """q|k projection for m-tile m, x chunk ch: [128, 256] PSUM,
                then 4 partition-shifted copies into DoubleRow layout."""
                s = st[r]
                pq = psC.tile([128, NCH], F32, name="pq", tag="p1")
                x_t = s["x"][ch]
                wp = s["wqkP"][m % NP]
                off = (m // NP) * 128
                for i, k in enumerate(KARR):
                    nc.tensor.matmul(
                        pq[:],
                        wp[:, k * 256 + off:k * 256 + off + 128],
                        x_t[:, k * NCH:(k + 1) * NCH],
                        start=(i == 0), stop=(i == KT - 1),
                    )
                # GPSIMD cannot touch PSUM, so: one DVE cast to an fp8 SBUF
                # stage, then 4 partition-shifted SBUF->SBUF copies scatter
                # it into the DoubleRow layout, split across Pool and DVE
                # per the repack_engines mode for this m-tile
                stage = lpool.tile([128, NCH], F8, name="qs", tag="qs",
                                   bufs=4)
                nc.vector.tensor_copy(stage[:], pq[:])
                qk = s["qk8"][m]
                csl = slice(ch * NCH, (ch + 1) * NCH)
                mode = repack_engines[m]
                for hh in range(2):
                    if mode == "split":
                        eng = nc.gpsimd if hh == 0 else nc.vector
                    else:
                        eng = nc.gpsimd if mode == "pool" else nc.vector
                    for t in range(2):
                        eng.tensor_copy(
                            qk[32 * hh:32 * hh + 32, t, csl],
                            stage[hh * DH + 32 * t:hh * DH + 32 * (t + 1), :],
                        )

            def emit_pv(r, ch, mt, c):
                """v projection for j-tile 2ch+mt, pair c: [128, 128] PSUM,
                copied around the ones block of the pair's v tile."""
                s = st[r]
                x_t = s["x"][ch]
                j = ch * (NCH // 128) + mt
                pv = psC.tile([128, 128], F32, name="pv", tag="p1")
                for k in range(KT):
                    nc.tensor.matmul(
                        pv[:],
                        x_t[:, k * NCH + mt * 128:k * NCH + (mt + 1) * 128],
                        s["wvP"][c][:, k * 128:(k + 1) * 128],
                        start=(k == 0), stop=(k == KT - 1),
                    )
                base = j * 3 * DH
                vt = s["v"][c]
                nc.vector.tensor_copy(vt[:, base:base + DH], pv[:, 0:DH])
                nc.vector.tensor_copy(vt[:, base + 2 * DH:base + 3 * DH],
                                      pv[:, DH:2 * DH])

            # ---------------- cell-stream emission ----------------
            def emit_scores(r, p, ic, jt):
                s = st[r]
                isl = slice(ic * ICH, (ic + 1) * ICH)
                s_ps = psA.tile([128, 2 * ICH], F32, name="s_ps", tag="s")
                for half in range(2):
                    po = 32 * half
                    q8 = s["qk8"][p]
                    k8 = s["qk8"][NP + p]
                    nc.tensor.matmul(
                        s_ps[:, half * ICH:(half + 1) * ICH],
                        k8[po:po + 32, :, jt * 128:(jt + 1) * 128],
                        q8[po:po + 32, :, isl],
                        start=True, stop=True,
                        perf_mode=PM,
                    )
                ex = epool.tile([128, 2 * ICH], BF16, name="ex", tag="ex")
                nc.scalar.activation(ex[:], s_ps[:], AF.Exp, scale=0.125)
                return ex

            def emit_av(r, p, jt, ex, av2):
                s = st[r]
                for half in range(2):
                    base = jt * 3 * DH + half * DH
                    vl = s["v"][p][:, base:base + 2 * DH]
                    nc.tensor.matmul(
                        av2[:, half * ICH:(half + 1) * ICH],
                        vl,
                        ex[:, half * ICH:(half + 1) * ICH],
                        start=(jt == 0), stop=(jt == NJT - 1),
                    )

            def emit_normalize(r, p, ic, av2, last=False):
                s = st[r]
                # one fast copy releases the PSUM accumulator for the next
                # stream; the reciprocal/mul chain runs off-critical-path.
                # For the rep's last stream no one needs the bank: skip the
                # copy and normalize straight from PSUM.
                if last:
                    avs = av2
                else:
                    avs = lpool.tile([128, 2 * ICH], F32, name="avs",
                                     tag="avs", bufs=2)
                    nc.vector.tensor_copy(avs[:], av2[:])
                for half in range(2):
                    l = 2 * p + half
                    ct, coff = l // 2, (l % 2) * DH
                    ao = s["aoT"].get((ct, ic))
                    if ao is None:
                        ao = aopool.tile([128, ICH], BF16,
                                         name=f"aoT{ct}_{ic}")
                        s["aoT"][(ct, ic)] = ao
                    # even head: rows [out | sums]; odd head: [sums | out]
                    o0, s0 = (0, DH) if half == 0 else (DH, 0)
                    hsl = slice(half * ICH, (half + 1) * ICH)
                    # rc rows sit at the out-rows' base partition: the DVE
                    # mul requires equal base partitions for two SBUF inputs.
                    # The recip/mul chain runs on GPSIMD (all-SBUF, legal
                    # there) to unload DVE — except last=True, where avs is
                    # still in PSUM, which GPSIMD cannot read.
                    eng = nc.vector if last else nc.gpsimd
                    rc = lpool.tile([128, ICH], F32, name="rc", tag="rc",
                                    bufs=2)
                    nc.vector.reciprocal(rc[o0:o0 + DH, :],
                                         avs[s0:s0 + DH, hsl])
                    eng.tensor_mul(
                        ao[coff:coff + DH, :],
                        avs[o0:o0 + DH, hsl],
                        rc[o0:o0 + DH, :],
                    )

            def emit_ph3(r, ic, lnt, h, drain=False):
                s = st[r]
                nt = ic * NTPI + lnt
                po = psC.tile([128, 512], F32, name="po", tag="p1")
                for c in range(CW // 128):
                    nc.tensor.matmul(
                        po[:],
                        s["aoT"][(c, ic)][:, lnt * 128:(lnt + 1) * 128],
                        s["woT"][:, c * DM + h * 512:c * DM + (h + 1) * 512],
                        start=(c == 0), stop=(c == CW // 128 - 1),
                    )
                os_ = lpool.tile([128, 512], F32, name="os", tag="os",
                                 bufs=3)
                if drain:
                    # tail units: the exp stream is done, so the idle ACT
                    # engine does the PSUM->SBUF copy (Copy shares the exp
                    # act table — no reload)
                    nc.scalar.activation(os_[:], po[:], AF.Copy)
                else:
                    nc.vector.tensor_copy(os_[:], po[:])
                nc.sync.dma_start(
                    out[nt * 128:(nt + 1) * 128, h * 512:(h + 1) * 512],
                    os_[:],
                )

            # ---------------- background unit queue ----------------
            def cell_order():
                # p-major with per-block i-chunk rotation: ph3(ic) needs all
                # pairs' norms at ic; rotation staggers each ic's completion
                # across block boundaries instead of piling all out-projection
                # work after the last block
                return [(p, (ic + p) % NIC, jt) for p in range(NP)
                        for ic in range(NIC) for jt in range(NJT)]

            def cell_deps(r, p, ic, jt):
                # pv last: its matmuls wait on wvT, which arrives after the
                # score-critical tensors — don't head-of-line block the q/k
                # units behind it in the PE queue
                return [
                    (r, "pq", p, 2 * ic),
                    (r, "pq", NP + p, jt // 2),
                    (r, "pq", p, 2 * ic + 1),
                    (r, "pv", jt // 2, jt % 2, p),
                ]

            def build_queue(r):
                """Background units for rep r in first-need order.
                Entry: (cost_ns, fn, key)."""
                q = deque()
                queued = set()

                def add(cost, fn, key):
                    if key in queued:
                        return
                    queued.add(key)
                    q.append((cost, fn, key))

                def need_x(ch):
                    for c2 in range(min(ch + 2, NXC - 1) + 1):
                        add(0.0, (lambda c3=c2: emit_dma_x(r, c3)),
                            (r, "x", c2))

                if r == 0:
                    add(0.0, emit_warmup, (r, "warm"))
                add(0.0, (lambda: emit_alloc(r)), (r, "alloc"))
                add(0.0, (lambda: emit_dma_x(r, 0)), (r, "x", 0))
                add(0.0, (lambda: emit_dma_wqk(r, 0)), (r, "wqk", 0))
                add(0.0, (lambda: emit_dma_x(r, 1)), (r, "x", 1))
                add(0.0, (lambda: emit_dma_wv(r, 0)), (r, "wv", 0))
                for (p, ic, jt) in cell_order():
                    if (p, jt) == (0, 8):
                        add(0.0, (lambda: emit_dma_wo(r)), (r, "wo"))
                    for key in cell_deps(r, p, ic, jt):
                        if key in queued:
                            continue
                        kind = key[1]
                        if kind == "pq":
                            _, _, m, ch = key
                            add(0.0, (lambda m=m: emit_dma_wqk(r, m % NP)),
                                (r, "wqk", m % NP))
                            need_x(ch)
                            add(853.0,
                                (lambda m=m, ch=ch:
                                 emit_pq(r, m, ch)), key)
                        else:
                            _, _, ch, mt, c = key
                            add(0.0, (lambda c=c: emit_dma_wv(r, c)),
                                (r, "wv", c))
                            need_x(ch)
                            add(427.0,
                                (lambda ch=ch, mt=mt, c=c:
                                 emit_pv(r, ch, mt, c)), key)
                return q

            emitted = set()

            def emit_cells(r, fillers, ph3q, is_last_rep=False):
                """Cell stream for rep r. fillers: background queue
                (leftovers from rep r-1 then rep r+1's units). ph3q:
                out-projection units, popped with priority."""
                credit = 0.0

                def pop_filler(force=False):
                    nonlocal credit
                    if force:
                        q = fillers if fillers else ph3q
                    else:
                        q = ph3q if ph3q else fillers
                    if not q:
                        return False
                    cost = q[0][0]
                    if not force and credit < cost:
                        return False
                    ent = q.popleft()
                    ent[1]()
                    if len(ent) > 2:
                        emitted.add(ent[2])
                    if not force:
                        credit -= cost
                    return True

                def ensure(key):
                    while key not in emitted:
                        assert fillers, f"dep {key} not in queue"
                        ent = fillers.popleft()
                        ent[1]()
                        if len(ent) > 2:
                            emitted.add(ent[2])

                def pop_free():
                    # zero-cost entries (DMAs/allocs) at the front ride free
                    while fillers and fillers[0][0] == 0.0:
                        ent = fillers.popleft()
                        ent[1]()
                        if len(ent) > 2:
                            emitted.add(ent[2])

                cells = cell_order()
                av_cur = None
                pend = None  # (p, ic, jt, ex, av2)
                norm_done = {}

                def finish(cell):
                    nonlocal credit
                    p, ic, jt, ex, av2 = cell
                    emit_av(r, p, jt, ex, av2)
                    credit = min(credit + slack, credit_cap)
                    if jt == NJT - 1:
                        emit_normalize(r, p, ic, av2,
                                       last=(is_last_rep
                                             and (p, ic) == cells[-1][:2]))
                        norm_done[ic] = norm_done.get(ic, 0) + 1
                        if norm_done[ic] == NP:
                            drain = (is_last_rep
                                     and (p, ic) == cells[-1][:2])
                            for lnt in range(NTPI):
                                for h in range(DM // 512):
                                    ph3q.append(
                                        (853.0,
                                         lambda ic=ic, lnt=lnt, h=h,
                                         drain=drain:
                                         emit_ph3(r, ic, lnt, h, drain)))
                        # PE spacer while DVE copies the accumulator out
                        for _ in range(spacer):
                            pop_filler(force=True)
                    else:
                        pop_free()
                        while pop_filler():
                            pass

                for i, (p, ic, jt) in enumerate(cells):
                    if jt == 0:
                        av_cur = psB.tile([128, 2 * ICH], F32, name="av",
                                          tag="av")
                    for key in cell_deps(r, p, ic, jt):
                        ensure(key)
                    ex = emit_scores(r, p, ic, jt)
                    if pend is not None:
                        finish(pend)
                    pend = (p, ic, jt, ex, av_cur)
                    if jt == 10 and i + 6 < len(cells):
                        # force-prefetch the next stream's deps so its first
                        # scores don't stall on just-in-time projections and
                        # their repack copies at the stream boundary
                        for key in cell_deps(r, *cells[i + 6]):
                            ensure(key)
                finish(pend)
                return fillers, ph3q

            # ---------------- weave reps ----------------
            carry = deque()
            ph3q = deque()
            for r in range(reps):
                fillers = deque(carry)
                fillers.extend(build_queue(r))
                carry, ph3q = emit_cells(r, fillers, ph3q,
                                         is_last_rep=(r == reps - 1))
            while ph3q or carry:
                q = ph3q if ph3q else carry
                q.popleft()[1]()

    nc.finalize()
    return nc


def make_in_maps(inputs_np):
    bf16 = mybir.dt.np(BF16)
    x = np.ascontiguousarray(inputs_np["x"], dtype=np.float32)
    w_qkv = np.asarray(inputs_np["w_qkv"], dtype=np.float32)
    w_out = np.asarray(inputs_np["w_out"], dtype=np.float32)
    in_maps = []
    xp_cache = {}
    for core in range(8):
        b, g = divmod(core, 2)
        if b not in xp_cache:
            # pack x[b]^T as [chunk, partition, k, n] so device chunk loads
            # are single linear DMAs
            xp_cache[b] = np.ascontiguousarray(
                x[b].T.reshape(KT, 128, NXC, NCH).transpose(2, 1, 0, 3)
            ).reshape(NXC, 128, KT * NCH).astype(bf16)
        xTb = xp_cache[b]
        wq = w_qkv[:, g * CW:(g + 1) * CW]
        wk = w_qkv[:, DM + g * CW:DM + (g + 1) * CW]
        wv_ = w_qkv[:, 2 * DM + g * CW:2 * DM + (g + 1) * CW]

        # per-pair tile-major packs: block p touches only its own slice
        wqkP = np.empty((NP, 128, KT * 256), np.float32)
        wvP = np.empty((NP, 128, KT * 128), np.float32)
        for p in range(NP):
            a = wq[:, p * 128:(p + 1) * 128].reshape(KT, 128, 128)
            k_ = wk[:, p * 128:(p + 1) * 128].reshape(KT, 128, 128)
            wqkP[p] = np.concatenate([a, k_], axis=2).transpose(
                1, 0, 2).reshape(128, KT * 256)
            wvP[p] = wv_[:, p * 128:(p + 1) * 128].reshape(
                KT, 128, 128).transpose(1, 0, 2).reshape(128, KT * 128)

        wo_ = w_out[g * CW:(g + 1) * CW, :]
        in_maps.append({
            "xT": xTb,
            "wqk": wqkP.astype(bf16),
            "wv": wvP.astype(bf16),
            "wo": np.ascontiguousarray(
                wo_.reshape(CW // 128, 128, DM).transpose(1, 0, 2)
            ).reshape(128, (CW // 128) * DM).astype(bf16),
        })
    return in_maps


_NC_CACHE = {}


def _get_nc():
    if "nc" not in _NC_CACHE:
        _NC_CACHE["nc"] = build_nc()
    return _NC_CACHE["nc"]


def kernel(x, w_qkv, w_out, b_out):
    b_out = np.asarray(b_out, dtype=np.float32)
    nc = _get_nc()
    in_maps = make_in_maps({"x": x, "w_qkv": w_qkv, "w_out": w_out})
    res = run_bass_kernel_spmd(nc, in_maps, core_ids=list(range(8)))
    _NC_CACHE["last_result"] = res
    out = np.empty((B, N, DM), np.float32)
    for b in range(B):
        out[b] = res.results[2 * b]["out"] + res.results[2 * b + 1]["out"] + b_out
    return out
